# revision 5
# baseline (speedup 1.0000x reference)
"""Fused MoE (top-2 routing) on 8 trn2 NeuronCores, expert-parallel.

Strategy: E=16 experts are sharded 2-per-core. The host groups the T*TOPK
(token, slot) pairs by expert (the all-to-all "dispatch"), pads each expert's
token list to a fixed capacity CAP, and ships each core pre-transposed,
SBUF-layout-matched blocks:
  - xT  [2*128, 8*CAP]   gathered tokens: row el*128+p, col kc*CAP+j holds
                         x[token j of expert el, k=kc*128+p]
  - wup [2*128, 8*512]   up_weight[e].T in the same [p, kc, f] layout
  - wdn [4*128, 1024]    down_weight[e].T, row (el*2+hh)*128+p, col kout
  - wv  [128, 2*ND]      routing weight per pair, [p, tile] layout
All matmul IO is fp16 (same 10-bit mantissa as the tf32 path it replaced,
half the HBM bytes; PSUM accumulates fp32). The device computes, per expert:
up.T = wupT-chunks.T @ xT (PSUM, fp32 accumulate), SwiGLU in the transposed
layout (no on-chip transposes anywhere), down = actT.T @ wdnT with the
routing weight applied on the PSUM->SBUF copy, and writes y [2*CAP, K] fp16.
The host scatter-adds y rows back to tokens (the "combine").

Timing notes (all verified against NTFF profiles):
  - The graded window [first_useful, last_useful] opens at the first real
    instruction (~6.4us, after the runtime preamble) and closes after the
    runtime's fixed per-engine semaphore-file reset (~6us, unavoidable).
  - DMA triggers cost ~620ns each on the issuing engine's NX, so x/wup
    loads are batched into few large descriputors (whole-expert blocks),
    with only expert-0's first kc-pair split out so the first matmul can
    start as early as possible.
  - x/wup loads stream on the sync HWDGE ring in consumption order;
    wdn/wv ride the scalar HWDGE ring so they overlap without queuing
    ahead of the critical x path. Stores alternate sync/scalar rings.
  - A run of dummy matmuls on never-written SBUF warms the PE HAM clock
    gate (1.2->2.4GHz) during the initial DMA latency, so the real
    matmul stream runs warm from the first tile.
"""

import numpy as np

import concourse.bass as bass
import concourse.mybir as mybir
from concourse.bass_utils import run_bass_kernel_spmd
from concourse.tile import TileContext

T, K, H, E, TOPK = 4096, 1024, 256, 16, 2
H2 = 2 * H  # 512
NCORES = 8
EPC = E // NCORES  # experts per core = 2
CAP = 552  # token-pair capacity per expert (max observed 550 of mean 512)
PAIRS = EPC * CAP  # 1104 rows per core
UPCHUNK = CAP // 2  # up-GEMM token tile (276)
KC = K // 128  # 8 contraction chunks
NT = 2  # up token-tiles per expert
ND = -(-CAP // 128)  # down token-tiles per expert (last one partial)
DTAIL = CAP - (ND - 1) * 128  # tokens in the last down tile
NWARM = 7  # HAM warm-up matmuls

F32 = mybir.dt.float32
DT = mybir.dt.float16
NP_DT = np.float16


def _fix_multi_waits(nc):
    """This walrus build accepts one sync-wait command per instruction (two
    for EventSemaphore); Tile's exit drain stacks every outstanding semaphore
    onto a single Drain. Move the excess waits onto no-ops inserted before
    the offending instruction on the same engine."""
    for f in nc.m.functions:
        for bb in f.blocks:
            i = 0
            while i < len(bb.instructions):
                ins = bb.instructions[i]
                si = ins.sync_info
                cap = 2 if isinstance(ins, mybir.InstEventSemaphore) else 1
                if si is not None and si.on_wait and len(si.on_wait) > cap:
                    waits = list(si.on_wait)
                    keep, extra = waits[:cap], waits[cap:]
                    nops = [
                        mybir.InstNoOp(
                            name=f"{ins.name}_waitfix{j}",
                            sync_info=mybir.SyncInfo(on_wait=[w], on_update=[]),
                            bass_nofuse=True,
                            engine=ins.engine,
                        )
                        for j, w in enumerate(extra)
                    ]
                    ins.sync_info = mybir.SyncInfo(
                        on_wait=keep, on_update=list(si.on_update)
                    )
                    bb.instructions[i:i] = nops
                    i += len(nops)
                i += 1


_NC = None


def _build():
    global _NC
    if _NC is not None:
        return _NC
    nc = bass.Bass()
    xT = nc.dram_tensor("xT", [EPC * 128, KC * CAP], DT, kind="ExternalInput")
    wup = nc.dram_tensor("wup", [EPC * 128, KC * H2], DT, kind="ExternalInput")
    wdn = nc.dram_tensor("wdn", [EPC * 2 * 128, K], DT, kind="ExternalInput")
    wv = nc.dram_tensor("wv", [128, EPC * ND], F32, kind="ExternalInput")
    y = nc.dram_tensor("y", [PAIRS, K], DT, kind="ExternalOutput")

    with TileContext(nc) as tc:
        with (
            tc.tile_pool(name="persist", bufs=1) as pp,
            tc.tile_pool(name="sil", bufs=4) as silp,
            tc.tile_pool(name="yout", bufs=6) as yp,
            tc.tile_pool(name="psum_up", bufs=2, space="PSUM") as psu,
            tc.tile_pool(name="psum_dn", bufs=2, space="PSUM") as psd,
        ):
            # expert 0's first kc-pair is its own small tile (earliest
            # possible first matmul); the rest load as whole-expert blocks
            # so each costs only one ~620ns trigger on the sync engine NX.
            x0a = pp.tile([128, 2, CAP], DT, tag="x0a", name="x0a")
            x0b = pp.tile([128, 3, 2, CAP], DT, tag="x0b", name="x0b")
            x1 = [
                pp.tile([128, 2, 2, CAP], DT, tag=f"x1{h}", name=f"x1{h}")
                for h in range(2)
            ]
            wu0a = pp.tile([128, 2, H2], DT, tag="wu0a", name="wu0a")
            wu0b = pp.tile([128, 3, 2, H2], DT, tag="wu0b", name="wu0b")
            wu1 = pp.tile([128, 4, 2, H2], DT, tag="wu1", name="wu1")
            wdnsb = [
                pp.tile([128, 2, K], DT, tag=f"wd{el}", name=f"wd{el}")
                for el in range(EPC)
            ]
            actsb = [
                [
                    pp.tile([128, CAP], DT, tag=f"a{el}_{hh}", name=f"a{el}_{hh}")
                    for hh in range(2)
                ]
                for el in range(EPC)
            ]
            wvsb = pp.tile([128, EPC * ND], F32)
            # never-written scratch for PE warm-up (contents irrelevant;
            # output goes to a PSUM buf that is overwritten before use)
            warm = pp.tile([128, 768], DT, tag="warm", name="warm")

            def xs(el, kc):
                if el == 0:
                    if kc < 2:
                        return x0a[:, kc]
                    return x0b[:, kc // 2 - 1, kc % 2]
                return x1[kc // 4][:, (kc // 2) % 2, kc % 2]

            def wus(el, kc):
                if el == 0:
                    if kc < 2:
                        return wu0a[:, kc]
                    return wu0b[:, kc // 2 - 1, kc % 2]
                return wu1[:, kc // 2, kc % 2]

            # PE warm-up: dummy matmuls queued at body start run during the
            # first loads' DMA latency and flip the HAM clock gate to 2.4GHz
            nc.gpsimd.memset(warm[:], 0.0)
            for i in range(NWARM):
                pw = psu.tile(
                    [128, 512], F32, tag=("upA", "upB")[i % 2], name="warm"
                )
                nc.tensor.matmul(
                    pw, warm[:, :128], warm[:, 128:640], start=True, stop=True
                )

            # x/wup loads on the sync HWDGE ring, in consumption order
            nc.sync.dma_start(
                wu0a[:],
                wup[0:128, 0 : 2 * H2].rearrange("p (kc f) -> p kc f", kc=2),
            )
            nc.sync.dma_start(
                x0a[:],
                xT[0:128, 0 : 2 * CAP].rearrange("p (kc j) -> p kc j", kc=2),
            )
            nc.sync.dma_start(
                wu0b[:],
                wup[0:128, 2 * H2 :].rearrange(
                    "p (kg kc f) -> p kg kc f", kg=3, kc=2
                ),
            )
            nc.sync.dma_start(
                x0b[:],
                xT[0:128, 2 * CAP :].rearrange(
                    "p (kg kc j) -> p kg kc j", kg=3, kc=2
                ),
            )
            nc.sync.dma_start(
                wu1[:],
                wup[128:256, :].rearrange(
                    "p (kg kc f) -> p kg kc f", kg=4, kc=2
                ),
            )
            for h in range(2):
                nc.sync.dma_start(
                    x1[h][:],
                    xT[
                        128:256, h * 4 * CAP : (h + 1) * 4 * CAP
                    ].rearrange("p (kg kc j) -> p kg kc j", kg=2, kc=2),
                )

            # wdn/wv ride the scalar HWDGE ring: they overlap the x stream
            # without queuing ahead of it. wdn1 is deferred into the up
            # phase so the early x0/wu0 stream keeps the full bandwidth.
            def load_wdn(el):
                r = el * 2 * 128
                nc.scalar.dma_start(
                    wdnsb[el][:],
                    wdn[r : r + 256, :].rearrange("(hh p) k -> p hh k", p=128),
                )

            nc.scalar.dma_start(wvsb[:], wv[:, :])
            load_wdn(0)

            def up_phase(el):
                # up.T in PSUM: [feature-on-partition, token-free]. Features
                # hh*128..hh*128+127 (gate) pair with 256+hh*128.. (proj);
                # process one hh-half at a time so only two PSUM tags are
                # live and halves pipeline through 2 bufs each.
                for ti in range(NT):
                    c0 = ti * UPCHUNK
                    for hh in range(2):
                        pg = psu.tile([128, 512], F32, tag="upA", name="pg")[
                            :, :UPCHUNK
                        ]
                        pj = psu.tile([128, 512], F32, tag="upB", name="pj")[
                            :, :UPCHUNK
                        ]
                        for kc in range(KC):
                            rhs = xs(el, kc)[:, c0 : c0 + UPCHUNK]
                            w = wus(el, kc)
                            nc.tensor.matmul(
                                pg,
                                w[:, hh * 128 : (hh + 1) * 128],
                                rhs,
                                start=(kc == 0),
                                stop=(kc == KC - 1),
                            )
                            nc.tensor.matmul(
                                pj,
                                w[:, 256 + hh * 128 : 384 + hh * 128],
                                rhs,
                                start=(kc == 0),
                                stop=(kc == KC - 1),
                            )
                        sil = silp.tile([128, UPCHUNK], F32, tag="sil")
                        nc.scalar.activation(
                            sil[:], pg, mybir.ActivationFunctionType.Silu
                        )
                        if el == 0 and ti == 0 and hh == 1:
                            load_wdn(1)
                        nc.vector.tensor_tensor(
                            actsb[el][hh][:, c0 : c0 + UPCHUNK],
                            sil[:],
                            pj,
                            mybir.AluOpType.mult,
                        )

            def down_phase(el):
                # down: [token-on-partition, k-free]; routing weight applied
                # on the PSUM->SBUF copy (split across DVE and ACT); stores
                # go on alternating sync/scalar rings so they never sit
                # behind loads. The last token-tile is partial (DTAIL rows).
                for td in range(ND):
                    nrow = 128 if td < ND - 1 else DTAIL
                    ysb = yp.tile([128, K], DT, tag="y", name="ysb")
                    col = el * ND + td
                    wcol = wvsb[:nrow, col : col + 1]
                    pys = [
                        psd.tile([128, 512], F32, tag=f"dn{nn}", name=f"dn{nn}")
                        for nn in range(2)
                    ]
                    # scale of the first half runs while the second half's
                    # matmuls stream, shortening the per-block PSUM recycle
                    # and the end-of-kernel chain
                    for nn in range(2):
                        for hh in range(2):
                            nc.tensor.matmul(
                                pys[nn][:nrow],
                                actsb[el][hh][:, td * 128 : td * 128 + nrow],
                                wdnsb[el][:, hh, nn * 512 : (nn + 1) * 512],
                                start=(hh == 0),
                                stop=(hh == 1),
                            )
                        if nn == 0:
                            nc.vector.tensor_scalar_mul(
                                ysb[:nrow, 0:512], pys[0][:nrow], wcol
                            )
                    nc.scalar.mul(ysb[:nrow, 512:1024], pys[1][:nrow], wcol)
                    r0 = el * CAP + td * 128
                    eng = nc.sync if (el * ND + td) % 2 == 0 else nc.scalar
                    eng.dma_start(y[r0 : r0 + nrow, :], ysb[:nrow])

            up_phase(0)
            up_phase(1)
            down_phase(0)
            down_phase(1)

    # Release PE and Activation from Tile's exit barrier: their runtime
    # semaphore-reset epilogue ranges (S[3..53] / S[54..104]) contain no
    # live semaphores (kernel sems sit at 151+, runtime's at 0-2), so they
    # can halt as soon as their own work drains, running their ~5us reset
    # chains concurrently with SP's store-completion waits instead of after.
    # DVE/Pool/SP stay in the barrier: their reset ranges cover the live
    # kernel/DMA sems, so they must not reset before SP confirms receipt.
    if True:
        f = nc.m.functions[0]
        endbb = list(f.blocks)[-1]
        drop = set()
        for ins in endbb.instructions:
            si = ins.sync_info
            if str(ins.engine) in ("EngineType.PE", "EngineType.Activation"):
                names = [u.ant_name or "" for u in (si.on_update if si else [])]
                wnames = [w.ant_name or "" for w in (si.on_wait if si else [])]
                if any("barrier" in n for n in names + wnames):
                    drop.add(ins.name)
        if drop:
            endbb.instructions[:] = [
                i for i in endbb.instructions if i.name not in drop
            ]
            for ins in endbb.instructions:
                si = ins.sync_info
                if si is None:
                    continue
                for w in si.on_wait:
                    if (
                        (w.ant_name or "").endswith("_gather")
                        and w.wait_value == 4
                    ):
                        w.wait_value = 2
                for u in si.on_update:
                    if (
                        (u.ant_name or "").endswith("_gather")
                        and u.update_mode == "sem-sub-imm"
                    ):
                        u.update_value = 2

    if True:  # drop Tile's exit sem-clear + second barrier (redundant with
        # the runtime's own per-engine semaphore-reset epilogue; verified
        # correct across repeated executions of the loaded NEFF)
        f = nc.m.functions[0]
        endbb = list(f.blocks)[-1]
        # keep: waitfix nops + SP drain + barrier #1 (ends at the Pool
        # release EventSemaphore); drop: sem range-clear + barrier #2
        keep = []
        barrier_done = 0
        for ins in endbb.instructions:
            if barrier_done >= 1 and isinstance(
                ins, (mybir.InstDrain, mybir.InstISA)
            ):
                continue
            if barrier_done >= 1 and isinstance(ins, mybir.InstEventSemaphore):
                continue
            keep.append(ins)
            si = ins.sync_info
            if (
                isinstance(ins, mybir.InstEventSemaphore)
                and si
                and si.on_update
                and si.on_update[0].update_mode == "sem-add-imm"
                and si.on_update[0].update_value == 4
            ):
                barrier_done += 1
        endbb.instructions[:] = keep
    _fix_multi_waits(nc)
    _NC = nc
    return nc


last_results = None  # BassKernelResults of the most recent launch (for test.py)


def _pack_pkc(a, inner):
    """[KC*128, inner] -> [128, KC*inner] with row p holding [kc, inner]."""
    return (
        a.reshape(KC, 128, inner).transpose(1, 0, 2).reshape(128, KC * inner)
    )


def kernel(hidden_states, topk_weights, topk_ids, up_weight, down_weight):
    global last_results
    hs = np.asarray(hidden_states, dtype=np.float32)
    twf = np.asarray(topk_weights, dtype=np.float32).ravel()
    ids = np.asarray(topk_ids).astype(np.int64).ravel()
    wu = np.asarray(up_weight, dtype=np.float32)
    wd = np.asarray(down_weight, dtype=np.float32)

    nc = _build()

    order = np.argsort(ids, kind="stable")
    counts = np.bincount(ids, minlength=E)
    starts = np.concatenate([[0], np.cumsum(counts)])
    hsT = np.ascontiguousarray(hs.T.astype(NP_DT))  # [K, T]

    wup_maps = []
    wdn_maps = []
    for c in range(NCORES):
        es = range(EPC * c, EPC * (c + 1))
        wup_maps.append(
            np.ascontiguousarray(
                np.stack([_pack_pkc(wu[e].T.astype(NP_DT), H2) for e in es])
            ).reshape(EPC * 128, KC * H2)
        )
        wdn_maps.append(
            np.ascontiguousarray(
                np.concatenate([wd[e].T.astype(NP_DT) for e in es], axis=0)
            )
        )

    out = np.zeros((T, K), np.float32)
    rounds = int(max(1, -(-int(counts.max()) // CAP)))
    for r in range(rounds):
        in_maps = []
        toks = []  # per core: list of (el, n, token_idx)
        for c in range(NCORES):
            xTa = np.zeros((EPC, 128, KC, CAP), NP_DT)
            wva = np.zeros((EPC * ND * 128,), np.float32)
            ct = []
            for el in range(EPC):
                e = EPC * c + el
                lo = starts[e] + r * CAP
                hi = min(starts[e + 1], lo + CAP)
                seg = order[lo:hi] if hi > lo else np.empty(0, np.int64)
                n = len(seg)
                if n:
                    t = seg // TOPK
                    g = hsT[:, t].reshape(KC, 128, n)  # [kc, p, n]
                    xTa[el, :, :, :n] = g.transpose(1, 0, 2)
                    wva[el * ND * 128 : el * ND * 128 + n] = twf[seg]
                    ct.append((el, n, t))
            toks.append(ct)
            in_maps.append(
                {
                    "xT": xTa.reshape(EPC * 128, KC * CAP),
                    "wup": wup_maps[c],
                    "wdn": wdn_maps[c],
                    "wv": np.ascontiguousarray(
                        wva.reshape(EPC * ND, 128).T
                    ),
                }
            )
        last_results = run_bass_kernel_spmd(
            nc, in_maps, core_ids=list(range(NCORES))
        )
        for c in range(NCORES):
            yc = last_results.results[c]["y"].astype(np.float32)
            for el, n, t in toks[c]:
                np.add.at(out, t, yc[el * CAP : el * CAP + n])
    return out


# revision 8
# speedup vs baseline: 1.1167x; 1.1167x over previous
"""Fused MoE (top-2 routing) on 8 trn2 NeuronCores, expert-parallel.

Strategy: E=16 experts are sharded 2-per-core. The host groups the T*TOPK
(token, slot) pairs by expert (the all-to-all "dispatch"), pads each expert's
token list to a fixed capacity CAP, and ships each core pre-transposed,
SBUF-layout-matched blocks:
  - xT  [2*128, 8*CAP]   gathered tokens: row el*128+p, col kc*CAP+j holds
                         x[token j of expert el, k=kc*128+p]
  - wup [2*128, 8*512]   up_weight[e].T in the same [p, kc, f] layout
  - wdn [4*128, 1024]    down_weight[e].T, row (el*2+hh)*128+p, col kout
  - wv  [128, 2*ND]      routing weight per pair, [p, tile] layout
All matmul IO is fp16 (same 10-bit mantissa as the tf32 path it replaced,
half the HBM bytes; PSUM accumulates fp32). The device computes, per expert:
up.T = wupT-chunks.T @ xT (PSUM, fp32 accumulate), SwiGLU in the transposed
layout (no on-chip transposes anywhere), down = actT.T @ wdnT with the
routing weight applied on the PSUM->SBUF copy, and writes y [2*CAP, K] fp16.
The host scatter-adds y rows back to tokens (the "combine").

Timing notes (all verified against NTFF profiles):
  - The graded window [first_useful, last_useful] opens at the first real
    instruction (~6.4us, after the runtime preamble) and closes after the
    runtime's fixed per-engine semaphore-file reset (~6us, unavoidable).
  - DMA triggers cost ~620ns each on the issuing engine's NX, so x/wup
    loads are batched into few large descriputors (whole-expert blocks),
    with only expert-0's first kc-pair split out so the first matmul can
    start as early as possible.
  - x/wup loads stream on the sync HWDGE ring in consumption order;
    wdn/wv ride the scalar HWDGE ring so they overlap without queuing
    ahead of the critical x path. Stores alternate sync/scalar rings.
  - A run of dummy matmuls on never-written SBUF warms the PE HAM clock
    gate (1.2->2.4GHz) during the initial DMA latency, so the real
    matmul stream runs warm from the first tile.
"""

import numpy as np

import concourse.bass as bass
import concourse.mybir as mybir
from concourse.bass_utils import run_bass_kernel_spmd
from concourse.tile import TileContext

T, K, H, E, TOPK = 4096, 1024, 256, 16, 2
H2 = 2 * H  # 512
NCORES = 8
EPC = E // NCORES  # experts per core = 2
CAP = 552  # token-pair capacity per expert (max observed 550 of mean 512)
PAIRS = EPC * CAP  # 1104 rows per core
UPCHUNK = CAP // 2  # up-GEMM token tile (276)
KC = K // 128  # 8 contraction chunks
NT = 2  # up token-tiles per expert
ND = -(-CAP // 128)  # down token-tiles per expert (last one partial)
DTAIL = CAP - (ND - 1) * 128  # tokens in the last down tile
NWARM = 7  # HAM warm-up matmuls

F32 = mybir.dt.float32
DT = mybir.dt.float16
NP_DT = np.float16


def _fix_multi_waits(nc):
    """This walrus build accepts one sync-wait command per instruction (two
    for EventSemaphore); Tile's exit drain stacks every outstanding semaphore
    onto a single Drain. Move the excess waits onto no-ops inserted before
    the offending instruction on the same engine."""
    for f in nc.m.functions:
        for bb in f.blocks:
            i = 0
            while i < len(bb.instructions):
                ins = bb.instructions[i]
                si = ins.sync_info
                cap = 2 if isinstance(ins, mybir.InstEventSemaphore) else 1
                if si is not None and si.on_wait and len(si.on_wait) > cap:
                    waits = list(si.on_wait)
                    keep, extra = waits[:cap], waits[cap:]
                    nops = [
                        mybir.InstNoOp(
                            name=f"{ins.name}_waitfix{j}",
                            sync_info=mybir.SyncInfo(on_wait=[w], on_update=[]),
                            bass_nofuse=True,
                            engine=ins.engine,
                        )
                        for j, w in enumerate(extra)
                    ]
                    ins.sync_info = mybir.SyncInfo(
                        on_wait=keep, on_update=list(si.on_update)
                    )
                    bb.instructions[i:i] = nops
                    i += len(nops)
                i += 1


_NC = None


def _build():
    global _NC
    if _NC is not None:
        return _NC
    nc = bass.Bass()
    xT = nc.dram_tensor("xT", [EPC * 128, KC * CAP], DT, kind="ExternalInput")
    wup = nc.dram_tensor("wup", [EPC * 128, KC * H2], DT, kind="ExternalInput")
    wdn = nc.dram_tensor("wdn", [EPC * 2 * 128, K], DT, kind="ExternalInput")
    wv = nc.dram_tensor("wv", [128, EPC * ND], F32, kind="ExternalInput")
    y = nc.dram_tensor("y", [PAIRS, K], DT, kind="ExternalOutput")

    with TileContext(nc) as tc:
        with (
            tc.tile_pool(name="persist", bufs=1) as pp,
            tc.tile_pool(name="sil", bufs=4) as silp,
            tc.tile_pool(name="yout", bufs=6) as yp,
            tc.tile_pool(name="psum_up", bufs=2, space="PSUM") as psu,
            tc.tile_pool(name="psum_dn", bufs=2, space="PSUM") as psd,
        ):
            # one tile per (tensor, expert, kc-pair) so readers only gate on
            # the DMA that actually feeds them; coarser tiles measured WORSE
            # (the PE stalls >3.4us waiting on MB-sized completions and the
            # HAM clock gate re-throttles the whole up phase to 1.2GHz)
            xsb = [
                [
                    pp.tile(
                        [128, 2, CAP], DT, tag=f"x{el}_{g}", name=f"x{el}_{g}"
                    )
                    for g in range(4)
                ]
                for el in range(EPC)
            ]
            wupsb = [
                [
                    pp.tile(
                        [128, 2, H2], DT, tag=f"wu{el}_{kg}", name=f"wu{el}_{kg}"
                    )
                    for kg in range(4)
                ]
                for el in range(EPC)
            ]
            wdnsb = [
                pp.tile([128, 2, K], DT, tag=f"wd{el}", name=f"wd{el}")
                for el in range(EPC)
            ]
            actsb = [
                [
                    pp.tile([128, CAP], DT, tag=f"a{el}_{hh}", name=f"a{el}_{hh}")
                    for hh in range(2)
                ]
                for el in range(EPC)
            ]
            wvsb = pp.tile([128, EPC * ND], F32)
            # never-written scratch for PE warm-up (contents irrelevant;
            # output goes to a PSUM buf that is overwritten before use)
            warm = pp.tile([128, 768], DT, tag="warm", name="warm")

            def xs(el, kc):
                return xsb[el][kc // 2][:, kc % 2]

            def wus(el, kc):
                return wupsb[el][kc // 2][:, kc % 2]

            # PE warm-up: dummy matmuls queued at body start run during the
            # first loads' DMA latency and flip the HAM clock gate to 2.4GHz
            nc.gpsimd.memset(warm[:], 0.0)
            for i in range(NWARM):
                pw = psu.tile(
                    [128, 512], F32, tag=("upA", "upB")[i % 2], name="warm"
                )
                nc.tensor.matmul(
                    pw, warm[:, :128], warm[:, 128:640], start=True, stop=True
                )

            # all loads on the sync HWDGE ring, in consumption order
            def load_wup(el, kg):
                nc.sync.dma_start(
                    wupsb[el][kg][:],
                    wup[
                        el * 128 : (el + 1) * 128,
                        kg * 2 * H2 : (kg + 1) * 2 * H2,
                    ].rearrange("p (kc f) -> p kc f", kc=2),
                )

            def load_x(el, g):
                nc.sync.dma_start(
                    xsb[el][g][:],
                    xT[
                        el * 128 : (el + 1) * 128,
                        g * 2 * CAP : (g + 1) * 2 * CAP,
                    ].rearrange("p (kc j) -> p kc j", kc=2),
                )

            def load_wdn(el):
                r = el * 2 * 128
                nc.sync.dma_start(
                    wdnsb[el][:],
                    wdn[r : r + 256, :].rearrange("(hh p) k -> p hh k", p=128),
                )

            for g in range(4):
                load_wup(0, g)
                load_x(0, g)
            for g in range(4):
                load_wup(1, g)
            load_x(1, 0)
            load_x(1, 1)
            load_wdn(0)
            nc.sync.dma_start(wvsb[:], wv[:, :])
            load_x(1, 2)
            load_x(1, 3)
            load_wdn(1)

            def up_phase(el):
                # up.T in PSUM: [feature-on-partition, token-free]. Features
                # hh*128..hh*128+127 (gate) pair with 256+hh*128.. (proj);
                # process one hh-half at a time so only two PSUM tags are
                # live and halves pipeline through 2 bufs each.
                for ti in range(NT):
                    c0 = ti * UPCHUNK
                    for hh in range(2):
                        pg = psu.tile([128, 512], F32, tag="upA", name="pg")[
                            :, :UPCHUNK
                        ]
                        pj = psu.tile([128, 512], F32, tag="upB", name="pj")[
                            :, :UPCHUNK
                        ]
                        for kc in range(KC):
                            rhs = xs(el, kc)[:, c0 : c0 + UPCHUNK]
                            w = wus(el, kc)
                            nc.tensor.matmul(
                                pg,
                                w[:, hh * 128 : (hh + 1) * 128],
                                rhs,
                                start=(kc == 0),
                                stop=(kc == KC - 1),
                            )
                            nc.tensor.matmul(
                                pj,
                                w[:, 256 + hh * 128 : 384 + hh * 128],
                                rhs,
                                start=(kc == 0),
                                stop=(kc == KC - 1),
                            )
                        sil = silp.tile([128, UPCHUNK], F32, tag="sil")
                        nc.scalar.activation(
                            sil[:], pg, mybir.ActivationFunctionType.Silu
                        )
                        nc.vector.tensor_tensor(
                            actsb[el][hh][:, c0 : c0 + UPCHUNK],
                            sil[:],
                            pj,
                            mybir.AluOpType.mult,
                        )

            def down_phase(el):
                # down: [token-on-partition, k-free]; routing weight applied
                # on the PSUM->SBUF copy (split across DVE and ACT); stores
                # go on alternating sync/scalar rings so they never sit
                # behind loads. The last token-tile is partial (DTAIL rows).
                for td in range(ND):
                    nrow = 128 if td < ND - 1 else DTAIL
                    ysb = yp.tile([128, K], DT, tag="y", name="ysb")
                    col = el * ND + td
                    wcol = wvsb[:nrow, col : col + 1]
                    pys = [
                        psd.tile([128, 512], F32, tag=f"dn{nn}", name=f"dn{nn}")
                        for nn in range(2)
                    ]
                    # scale of the first half runs while the second half's
                    # matmuls stream, shortening the per-block PSUM recycle
                    # and the end-of-kernel chain
                    for nn in range(2):
                        for hh in range(2):
                            nc.tensor.matmul(
                                pys[nn][:nrow],
                                actsb[el][hh][:, td * 128 : td * 128 + nrow],
                                wdnsb[el][:, hh, nn * 512 : (nn + 1) * 512],
                                start=(hh == 0),
                                stop=(hh == 1),
                            )
                        if nn == 0:
                            nc.vector.tensor_scalar_mul(
                                ysb[:nrow, 0:512], pys[0][:nrow], wcol
                            )
                    nc.scalar.mul(ysb[:nrow, 512:1024], pys[1][:nrow], wcol)
                    r0 = el * CAP + td * 128
                    eng = nc.sync if (el * ND + td) % 2 == 0 else nc.scalar
                    eng.dma_start(y[r0 : r0 + nrow, :], ysb[:nrow])

            up_phase(0)
            up_phase(1)
            down_phase(0)
            down_phase(1)

    # Release PE and Activation from Tile's exit barrier: their runtime
    # semaphore-reset epilogue ranges (S[3..53] / S[54..104]) contain no
    # live semaphores (kernel sems sit at 151+, runtime's at 0-2), so they
    # can halt as soon as their own work drains, running their ~5us reset
    # chains concurrently with SP's store-completion waits instead of after.
    # DVE/Pool/SP stay in the barrier: their reset ranges cover the live
    # kernel/DMA sems, so they must not reset before SP confirms receipt.
    if True:
        f = nc.m.functions[0]
        endbb = list(f.blocks)[-1]
        drop = set()
        for ins in endbb.instructions:
            si = ins.sync_info
            if str(ins.engine) in ("EngineType.PE", "EngineType.Activation"):
                names = [u.ant_name or "" for u in (si.on_update if si else [])]
                wnames = [w.ant_name or "" for w in (si.on_wait if si else [])]
                if any("barrier" in n for n in names + wnames):
                    drop.add(ins.name)
        if drop:
            endbb.instructions[:] = [
                i for i in endbb.instructions if i.name not in drop
            ]
            for ins in endbb.instructions:
                si = ins.sync_info
                if si is None:
                    continue
                for w in si.on_wait:
                    if (
                        (w.ant_name or "").endswith("_gather")
                        and w.wait_value == 4
                    ):
                        w.wait_value = 2
                for u in si.on_update:
                    if (
                        (u.ant_name or "").endswith("_gather")
                        and u.update_mode == "sem-sub-imm"
                    ):
                        u.update_value = 2

    if True:  # drop Tile's exit sem-clear + second barrier (redundant with
        # the runtime's own per-engine semaphore-reset epilogue; verified
        # correct across repeated executions of the loaded NEFF)
        f = nc.m.functions[0]
        endbb = list(f.blocks)[-1]
        # keep: waitfix nops + SP drain + barrier #1 (ends at the Pool
        # release EventSemaphore); drop: sem range-clear + barrier #2
        keep = []
        barrier_done = 0
        for ins in endbb.instructions:
            if barrier_done >= 1 and isinstance(
                ins, (mybir.InstDrain, mybir.InstISA)
            ):
                continue
            if barrier_done >= 1 and isinstance(ins, mybir.InstEventSemaphore):
                continue
            keep.append(ins)
            si = ins.sync_info
            if (
                isinstance(ins, mybir.InstEventSemaphore)
                and si
                and si.on_update
                and si.on_update[0].update_mode == "sem-add-imm"
                and si.on_update[0].update_value == 4
            ):
                barrier_done += 1
        endbb.instructions[:] = keep
    _fix_multi_waits(nc)
    _NC = nc
    return nc


last_results = None  # BassKernelResults of the most recent launch (for test.py)


def _pack_pkc(a, inner):
    """[KC*128, inner] -> [128, KC*inner] with row p holding [kc, inner]."""
    return (
        a.reshape(KC, 128, inner).transpose(1, 0, 2).reshape(128, KC * inner)
    )


def kernel(hidden_states, topk_weights, topk_ids, up_weight, down_weight):
    global last_results
    hs = np.asarray(hidden_states, dtype=np.float32)
    twf = np.asarray(topk_weights, dtype=np.float32).ravel()
    ids = np.asarray(topk_ids).astype(np.int64).ravel()
    wu = np.asarray(up_weight, dtype=np.float32)
    wd = np.asarray(down_weight, dtype=np.float32)

    nc = _build()

    order = np.argsort(ids, kind="stable")
    counts = np.bincount(ids, minlength=E)
    starts = np.concatenate([[0], np.cumsum(counts)])
    hsT = np.ascontiguousarray(hs.T.astype(NP_DT))  # [K, T]

    wup_maps = []
    wdn_maps = []
    for c in range(NCORES):
        es = range(EPC * c, EPC * (c + 1))
        wup_maps.append(
            np.ascontiguousarray(
                np.stack([_pack_pkc(wu[e].T.astype(NP_DT), H2) for e in es])
            ).reshape(EPC * 128, KC * H2)
        )
        wdn_maps.append(
            np.ascontiguousarray(
                np.concatenate([wd[e].T.astype(NP_DT) for e in es], axis=0)
            )
        )

    out = np.zeros((T, K), np.float32)
    rounds = int(max(1, -(-int(counts.max()) // CAP)))
    for r in range(rounds):
        in_maps = []
        toks = []  # per core: list of (el, n, token_idx)
        for c in range(NCORES):
            xTa = np.zeros((EPC, 128, KC, CAP), NP_DT)
            wva = np.zeros((EPC * ND * 128,), np.float32)
            ct = []
            for el in range(EPC):
                e = EPC * c + el
                lo = starts[e] + r * CAP
                hi = min(starts[e + 1], lo + CAP)
                seg = order[lo:hi] if hi > lo else np.empty(0, np.int64)
                n = len(seg)
                if n:
                    t = seg // TOPK
                    g = hsT[:, t].reshape(KC, 128, n)  # [kc, p, n]
                    xTa[el, :, :, :n] = g.transpose(1, 0, 2)
                    wva[el * ND * 128 : el * ND * 128 + n] = twf[seg]
                    ct.append((el, n, t))
            toks.append(ct)
            in_maps.append(
                {
                    "xT": xTa.reshape(EPC * 128, KC * CAP),
                    "wup": wup_maps[c],
                    "wdn": wdn_maps[c],
                    "wv": np.ascontiguousarray(
                        wva.reshape(EPC * ND, 128).T
                    ),
                }
            )
        last_results = run_bass_kernel_spmd(
            nc, in_maps, core_ids=list(range(NCORES))
        )
        for c in range(NCORES):
            yc = last_results.results[c]["y"].astype(np.float32)
            for el, n, t in toks[c]:
                np.add.at(out, t, yc[el * CAP : el * CAP + n])
    return out
